# revision 13
# baseline (speedup 1.0000x reference)
"""Trainium2 Bass kernel for tf-idf weighted embedding pooling + MLP.

Math: per batch row b (64 rows), tf[b,s] = within-row count of token x[s,b];
scores = where(tok==0, 0, tf*idf[tok]); pooled[b] = sum_s scores * emb[tok];
out = softmax(relu(relu(pooled@W1.T+b1)@W2.T+b2)@W3.T+b3).

Key identity: pooled[b] = sum_t c_t^2 * idf_t * emb_t  (c_t = count of t in row b).
Histogram per row is computed on the PE as a one-hot digit matmul:
  tok = hi*256 + lo,  H[lo, hi] = OneHotLo^T @ OneHotHi  (accumulated over s)
so H[lo, hi] = count of token hi*256+lo.  Then a[lo, hi] = H^2 * idfT[lo, hi].

Phase 2 (default): vocab-sharded pooled matmul. Each core histograms its own
8 batch rows, AllToAll redistributes the a-vectors so core c holds a[all 64
rows, vocab shard c], each core contracts its 6400-row emb shard (zero-padded
past 50000) against a for all 64 rows, and a ReduceScatter hands core c the
final pooled rows 8c..8c+8 for its MLP slice. Per-core HBM read of emb drops
from 51.2MB to 6.4MB.

Phase 1 (fallback): every core streams the full 51.2MB emb table.
"""

import os
import sys

import numpy as np

sys.path.insert(0, "/opt/trn_rl_repo")

import concourse.bass as bass  # noqa: E402,F401
import concourse.mybir as mybir  # noqa: E402
import concourse.tile as tile  # noqa: E402
from concourse import bacc  # noqa: E402
from concourse.masks import make_identity  # noqa: E402

P = 128
S = 2048
B = 64
D = 256
V = 50000
NCORES = 8
RPC = B // NCORES  # rows per core
NHI = 196  # ceil(50000/256)
NLO = 256
STILES = S // P  # 16
VPAD = NHI * NLO  # 50176
NHL = 25  # hi rows per vocab shard (200 padded hi rows / 8)
VSH = NHL * NLO  # 6400 vocab rows per shard

F32 = mybir.dt.float32
BF16 = mybir.dt.bfloat16
I32 = mybir.dt.int32

_CACHE = {}


def _mlp_tail(nc, tc, cpool, ps_mlp, pooled_sb, identity,
              w1t_sb, b1_sb, w2t_sb, b2a_sb, b2b_sb, w3a_sb, w3b_sb, b3_sb, out):
    """pooled_sb [RPC, 256] -> softmax out DMA."""
    pooledT = cpool.tile([P, 2, RPC], F32, tag="pooledT")
    for kc in range(2):
        ptp = ps_mlp.tile([P, RPC], F32, tag="ptp")
        nc.tensor.transpose(
            ptp[:, :], pooled_sb[:, kc * P : (kc + 1) * P], identity[:RPC, :RPC]
        )
        nc.vector.tensor_copy(pooledT[:, kc, :], ptp[:, :])

    h1_ps = ps_mlp.tile([100, RPC], F32, tag="h1")
    for kc in range(2):
        nc.tensor.matmul(
            h1_ps[:, :], lhsT=w1t_sb[:, kc, :], rhs=pooledT[:, kc, :],
            start=(kc == 0), stop=(kc == 1),
        )
    h1_sb = cpool.tile([100, RPC], F32, tag="h1_sb")
    nc.scalar.activation(
        h1_sb[:], h1_ps[:, :], mybir.ActivationFunctionType.Relu,
        bias=b1_sb[:, 0:1], scale=1.0,
    )

    h2a_ps = ps_mlp.tile([P, RPC], F32, tag="h2a")
    nc.tensor.matmul(h2a_ps[:, :], lhsT=w2t_sb[:, 0:128], rhs=h1_sb[:, :],
                     start=True, stop=True)
    h2b_ps = ps_mlp.tile([22, RPC], F32, tag="h2b")
    nc.tensor.matmul(h2b_ps[:, :], lhsT=w2t_sb[:, 128:150], rhs=h1_sb[:, :],
                     start=True, stop=True)
    h2a_sb = cpool.tile([P, RPC], F32, tag="h2a_sb")
    h2b_sb = cpool.tile([22, RPC], F32, tag="h2b_sb")
    nc.scalar.activation(h2a_sb[:], h2a_ps[:, :],
                         mybir.ActivationFunctionType.Relu,
                         bias=b2a_sb[:, 0:1], scale=1.0)
    nc.scalar.activation(h2b_sb[:], h2b_ps[:, :],
                         mybir.ActivationFunctionType.Relu,
                         bias=b2b_sb[:, 0:1], scale=1.0)

    lg_ps = ps_mlp.tile([2, RPC], F32, tag="lg")
    nc.tensor.matmul(lg_ps[:, :], lhsT=w3a_sb[:, :], rhs=h2a_sb[:, :],
                     start=True, stop=False)
    nc.tensor.matmul(lg_ps[:, :], lhsT=w3b_sb[:, :], rhs=h2b_sb[:, :],
                     start=False, stop=True)
    lg_sb = cpool.tile([2, RPC], F32, tag="lg_sb")
    nc.scalar.add(lg_sb[:], lg_ps[:, :], b3_sb[:, 0:1])

    lt_ps = ps_mlp.tile([RPC, 2], F32, tag="lt")
    nc.tensor.transpose(lt_ps[:, :], lg_sb[:, :], identity[:2, :2])
    e_sb = cpool.tile([RPC, 2], F32, tag="e_sb")
    nc.scalar.activation(e_sb[:], lt_ps[:, :], mybir.ActivationFunctionType.Exp)
    ssum = cpool.tile([RPC, 1], F32, tag="ssum")
    nc.vector.tensor_reduce(ssum[:], e_sb[:], axis=mybir.AxisListType.X,
                            op=mybir.AluOpType.add)
    rinv = cpool.tile([RPC, 1], F32, tag="rinv")
    nc.vector.reciprocal(rinv[:], ssum[:])
    res_sb = cpool.tile([RPC, 2], F32, tag="res_sb")
    nc.vector.tensor_scalar(out=res_sb[:], in0=e_sb[:], scalar1=rinv[:, 0:1],
                            scalar2=None, op0=mybir.AluOpType.mult)
    nc.sync.dma_start(out[:, :], res_sb[:])


def _build_nc3(reps=1):
    """Phase 3: vocab-sharded pooled matmul with DMA-friendly layouts.

    Per core: histogram own 8 rows via one-hot matmuls (both one-hots on
    DVE via is_equal), a = (H*sqrt(idf))^2 in bf16, AllToAll redistributes
    so core c holds a[all 64 rows, vocab shard c] with lo on partitions,
    bf16 emb shard (cast during SWDGE DMA) contracts against a, then a
    ReduceScatter yields each core's 8 pooled rows for the MLP tail.

    a2a layout [g, p(lo), mh, hl, r] keeps per-descriptor runs at 400B
    (vs 32B in phase 2), and the receive side loads one [128, 8*2*25*8]
    tile whose [p, j, mh, hl, r] slices feed matmul lhsT APs directly.
    """
    nc = bacc.Bacc(None, target_bir_lowering=False, debug=False)

    NHIP = 200  # hi padded to 8*25

    xt = nc.dram_tensor("xt", [RPC, S], I32, kind="ExternalInput")
    # emb shard pre-scaled by idf and cast to bf16 on the host
    embs = nc.dram_tensor("embs", [VSH, D], BF16, kind="ExternalInput")
    w1t = nc.dram_tensor("w1t", [256, 100], F32, kind="ExternalInput")
    b1 = nc.dram_tensor("b1", [100], F32, kind="ExternalInput")
    w2t = nc.dram_tensor("w2t", [100, 150], F32, kind="ExternalInput")
    b2 = nc.dram_tensor("b2", [150], F32, kind="ExternalInput")
    w3t = nc.dram_tensor("w3t", [150, 2], F32, kind="ExternalInput")
    b3 = nc.dram_tensor("b3", [2], F32, kind="ExternalInput")
    out = nc.dram_tensor("out", [RPC, 2], F32, kind="ExternalOutput")

    with tile.TileContext(nc) as tc:
        with (
            tc.tile_pool(name="const", bufs=1) as cpool,
            tc.tile_pool(name="work", bufs=3) as wpool,
            tc.tile_pool(name="oh", bufs=6) as ohpool,
            tc.tile_pool(name="embp", bufs=2) as embpool,
            tc.tile_pool(name="arp", bufs=2) as arpool,
            tc.tile_pool(name="dram", bufs=2, space="DRAM") as dpool,
            tc.tile_pool(name="ps_acc", bufs=2, space="PSUM") as ps_acc,
        ):
            # ---------- constants ----------
            iota_i32 = cpool.tile([P, NLO], I32)
            nc.gpsimd.iota(iota_i32[:], pattern=[[1, NLO]], base=0,
                           channel_multiplier=0)
            iota_bf = cpool.tile([P, NLO], BF16)
            nc.vector.tensor_copy(iota_bf[:], iota_i32[:])

            identity = cpool.tile([P, P], F32)
            make_identity(nc, identity[:])

            w1t_sb = cpool.tile([P, 2, 100], F32)
            nc.sync.dma_start(w1t_sb[:, :, :],
                              w1t[:, :].rearrange("(c p) m -> p c m", p=P))
            b1_sb = cpool.tile([100, 1], F32)
            nc.sync.dma_start(b1_sb[:, :], b1[:, None])
            w2t_sb = cpool.tile([100, 150], F32)
            nc.sync.dma_start(w2t_sb[:, :], w2t[:, :])
            b2a_sb = cpool.tile([128, 1], F32)
            b2b_sb = cpool.tile([22, 1], F32)
            nc.sync.dma_start(b2a_sb[:, :], b2[:128, None])
            nc.sync.dma_start(b2b_sb[:, :], b2[128:150, None])
            w3a_sb = cpool.tile([128, 2], F32)
            w3b_sb = cpool.tile([22, 2], F32)
            nc.sync.dma_start(w3a_sb[:, :], w3t[0:128, :])
            nc.sync.dma_start(w3b_sb[:, :], w3t[128:150, :])
            b3_sb = cpool.tile([2, 1], F32)
            nc.sync.dma_start(b3_sb[:, :], b3[:, None])

            for _rep in range(reps):
                # ---------- emb shard (idf-scaled bf16) ----------
                embc = embpool.tile([P, 50, D], BF16, tag="embc")
                nc.sync.dma_start(
                    embc[:, :, :],
                    embs[:, :].rearrange("(c p) d -> p c d", p=P))

                # ---------- tokens: [128, RPC*16], s = p*16 + f ----------
                tok_i32 = cpool.tile([P, RPC * STILES], I32, tag="tok", bufs=2)
                for r in range(RPC):
                    nc.sync.dma_start(
                        tok_i32[:, r * STILES : (r + 1) * STILES],
                        xt[r, :].rearrange("(p f) -> p f", p=P),
                    )
                lo_i32 = wpool.tile([P, RPC * STILES], I32, tag="lo_i32")
                hi_i32 = wpool.tile([P, RPC * STILES], I32, tag="hi_i32")
                nc.vector.tensor_scalar(
                    out=lo_i32[:], in0=tok_i32[:], scalar1=255, scalar2=None,
                    op0=mybir.AluOpType.bitwise_and)
                nc.vector.tensor_scalar(
                    out=hi_i32[:], in0=tok_i32[:], scalar1=8, scalar2=None,
                    op0=mybir.AluOpType.logical_shift_right)
                lo_f = cpool.tile([P, RPC * STILES], F32, tag="lo_f", bufs=2)
                hi_f = cpool.tile([P, RPC * STILES], F32, tag="hi_f", bufs=2)
                nc.vector.tensor_copy(lo_f[:], lo_i32[:])
                nc.vector.tensor_copy(hi_f[:], hi_i32[:])

                # ---------- per-row histograms -> a = (H*sqrt(idf))^2 ------
                a_all = [cpool.tile([P, NHIP, RPC], BF16, name=f"a_all{mh}",
                                    tag=f"a_all{mh}", bufs=2)
                         for mh in range(2)]
                a2a_in = dpool.tile([8, P, 2, NHL, RPC], BF16, tag="a2a_in")
                a2a_out = dpool.tile([8, P, 2, NHL, RPC], BF16, tag="a2a_out")

                with tc.tile_pool(name="ps_ht", bufs=2, space="PSUM") as ps_ht:
                    for r in range(RPC):
                        ht_ps = [ps_ht.tile([P, NHIP], F32, name=f"ht{mh}",
                                            tag=f"ht{mh}")
                                 for mh in range(2)]
                        for f in range(STILES):
                            col = r * STILES + f
                            lo_oh = ohpool.tile([P, NLO], BF16, tag="lo_oh")
                            hi_oh = ohpool.tile([P, NHIP], BF16, tag="hi_oh")
                            nc.vector.tensor_scalar(
                                out=lo_oh[:], in0=iota_bf[:],
                                scalar1=lo_f[:, col : col + 1],
                                scalar2=None,
                                op0=mybir.AluOpType.is_equal)
                            nc.vector.tensor_scalar(
                                out=hi_oh[:], in0=iota_bf[:, :NHIP],
                                scalar1=hi_f[:, col : col + 1],
                                scalar2=None,
                                op0=mybir.AluOpType.is_equal)
                            for mh in range(2):
                                nc.tensor.matmul(
                                    ht_ps[mh][:, :],
                                    lhsT=lo_oh[:, mh * P : (mh + 1) * P],
                                    rhs=hi_oh[:, :],
                                    start=(f == 0), stop=(f == STILES - 1))
                        for mh in range(2):
                            nc.scalar.square(a_all[mh][:, :, r],
                                             ht_ps[mh][:, :])

                # ---------- exchange a ----------
                for mh in range(2):
                    nc.sync.dma_start(
                        a2a_in[:, :, mh, :, :].rearrange(
                            "g p hl r -> p g hl r"),
                        a_all[mh][:, :, :].rearrange(
                            "p (g hl) r -> p g hl r", g=8))
                nc.gpsimd.collective_compute(
                    "AllToAll", mybir.AluOpType.bypass,
                    replica_groups=[list(range(NCORES))],
                    ins=[a2a_in[:, :, :, :, :]],
                    outs=[a2a_out[:, :, :, :, :]],
                )
                ar_full = arpool.tile([P, 8, 2, NHL, RPC], BF16, tag="ar")
                nc.sync.dma_start(
                    ar_full[:, :, :, :, :],
                    a2a_out[:, :, :, :, :].rearrange(
                        "j p mh hl r -> p j mh hl r"))
                # matmul lhsT needs one contiguous free dim: reorder so
                # (j, r) is innermost per k-block; split across DVE/ACT
                ar2 = arpool.tile([P, NHL, 2, 8, RPC], BF16, tag="ar2")
                nc.vector.tensor_copy(
                    ar2[:, :, 0, :, :],
                    ar_full[:, :, 0, :, :].rearrange(
                        "p j hl r -> p hl j r"))
                nc.scalar.copy(
                    ar2[:, :, 1, :, :],
                    ar_full[:, :, 1, :, :].rearrange(
                        "p j hl r -> p hl j r"))

                # ---------- pooled[64, 256] over own vocab shard ----------
                pooled_ps = ps_acc.tile([B, D], F32, tag="pooled")
                for k in range(50):
                    mh, hl = k & 1, k >> 1
                    nc.tensor.matmul(
                        pooled_ps[:, :],
                        lhsT=ar2[:, hl, mh, :, :],
                        rhs=embc[:, k, :],
                        start=(k == 0), stop=(k == 49))
                pooled_full = cpool.tile([B, D], F32, tag="pooled_full",
                                         bufs=2)
                nc.vector.tensor_copy(pooled_full[:], pooled_ps[:, :])
                rs_in = dpool.tile([B, D], F32, tag="rs_in")
                rs_out = dpool.tile([RPC, D], F32, tag="rs_out")
                nc.sync.dma_start(rs_in[:, :], pooled_full[:])
                nc.gpsimd.collective_compute(
                    "ReduceScatter", mybir.AluOpType.add,
                    replica_groups=[list(range(NCORES))],
                    ins=[rs_in[:, :]],
                    outs=[rs_out[:, :]],
                )
                pooled_sb = cpool.tile([RPC, D], F32, tag="pooled_sb")
                nc.sync.dma_start(pooled_sb[:], rs_out[:, :])

                # ---------- MLP + softmax on own 8 rows ----------
                with tc.tile_pool(name="ps_mlp", bufs=1,
                                  space="PSUM") as ps_mlp:
                    _mlp_tail(nc, tc, cpool, ps_mlp, pooled_sb, identity,
                              w1t_sb, b1_sb, w2t_sb, b2a_sb, b2b_sb,
                              w3a_sb, w3b_sb, b3_sb, out)

    nc.compile()
    return nc


def _build_nc(phase=1, reps=1):
    if phase == 3:
        return _build_nc3(reps)
    nc = bacc.Bacc(None, target_bir_lowering=False, debug=False)

    xt = nc.dram_tensor("xt", [RPC, S], I32, kind="ExternalInput")
    if phase == 1:
        emb = nc.dram_tensor("emb", [V, D], F32, kind="ExternalInput")
    else:
        embs = nc.dram_tensor("embs", [VSH, D], F32, kind="ExternalInput")
    idf_t = nc.dram_tensor("idf_t", [NLO, NHI], F32, kind="ExternalInput")
    w1t = nc.dram_tensor("w1t", [256, 100], F32, kind="ExternalInput")
    b1 = nc.dram_tensor("b1", [100], F32, kind="ExternalInput")
    w2t = nc.dram_tensor("w2t", [100, 150], F32, kind="ExternalInput")
    b2 = nc.dram_tensor("b2", [150], F32, kind="ExternalInput")
    w3t = nc.dram_tensor("w3t", [150, 2], F32, kind="ExternalInput")
    b3 = nc.dram_tensor("b3", [2], F32, kind="ExternalInput")
    out = nc.dram_tensor("out", [RPC, 2], F32, kind="ExternalOutput")

    with tile.TileContext(nc) as tc:
        with (
            tc.tile_pool(name="const", bufs=1) as cpool,
            tc.tile_pool(name="work", bufs=3) as wpool,
            tc.tile_pool(name="oh", bufs=6) as ohpool,
            tc.tile_pool(name="embp", bufs=17) as embpool,
            tc.tile_pool(name="arp", bufs=6) as arpool,
            tc.tile_pool(name="dram", bufs=2, space="DRAM") as dpool,
            tc.tile_pool(name="ps_acc", bufs=2, space="PSUM") as ps_acc,
        ):
            # ---------- constants ----------
            iota_i32 = cpool.tile([P, NLO], I32)
            nc.gpsimd.iota(iota_i32[:], pattern=[[1, NLO]], base=0,
                           channel_multiplier=0)
            iota_bf = cpool.tile([P, NLO], BF16)
            nc.vector.tensor_copy(iota_bf[:], iota_i32[:])

            identity = cpool.tile([P, P], F32)
            make_identity(nc, identity[:])

            idf_sb = cpool.tile([P, 2, NHI], F32)
            nc.sync.dma_start(idf_sb[:, 0, :], idf_t[0:128, :])
            nc.sync.dma_start(idf_sb[:, 1, :], idf_t[128:256, :])

            w1t_sb = cpool.tile([P, 2, 100], F32)
            nc.sync.dma_start(w1t_sb[:, :, :],
                              w1t[:, :].rearrange("(c p) m -> p c m", p=P))
            b1_sb = cpool.tile([100, 1], F32)
            nc.sync.dma_start(b1_sb[:, :], b1[:, None])
            w2t_sb = cpool.tile([100, 150], F32)
            nc.sync.dma_start(w2t_sb[:, :], w2t[:, :])
            b2a_sb = cpool.tile([128, 1], F32)
            b2b_sb = cpool.tile([22, 1], F32)
            nc.sync.dma_start(b2a_sb[:, :], b2[:128, None])
            nc.sync.dma_start(b2b_sb[:, :], b2[128:150, None])
            w3a_sb = cpool.tile([128, 2], F32)
            w3b_sb = cpool.tile([22, 2], F32)
            nc.sync.dma_start(w3a_sb[:, :], w3t[0:128, :])
            nc.sync.dma_start(w3b_sb[:, :], w3t[128:150, :])
            b3_sb = cpool.tile([2, 1], F32)
            nc.sync.dma_start(b3_sb[:, :], b3[:, None])

            for _rep in range(reps):
                # ---------- tokens: [128, RPC*16], s = p*16 + f per row ------
                tok_i32 = cpool.tile([P, RPC * STILES], I32, tag="tok", bufs=2)
                for r in range(RPC):
                    nc.sync.dma_start(
                        tok_i32[:, r * STILES : (r + 1) * STILES],
                        xt[r, :].rearrange("(p f) -> p f", p=P),
                    )
                lo_i32 = wpool.tile([P, RPC * STILES], I32, tag="lo_i32")
                hi_i32 = wpool.tile([P, RPC * STILES], I32, tag="hi_i32")
                nc.vector.tensor_scalar(
                    out=lo_i32[:], in0=tok_i32[:], scalar1=255, scalar2=None,
                    op0=mybir.AluOpType.bitwise_and)
                nc.vector.tensor_scalar(
                    out=hi_i32[:], in0=tok_i32[:], scalar1=8, scalar2=None,
                    op0=mybir.AluOpType.logical_shift_right)
                lo_f = cpool.tile([P, RPC * STILES], F32, tag="lo_f", bufs=2)
                hi_f = cpool.tile([P, RPC * STILES], F32, tag="hi_f", bufs=2)
                nc.vector.tensor_copy(lo_f[:], lo_i32[:])
                nc.vector.tensor_copy(hi_f[:], hi_i32[:])
                # negated hi for the ACT-engine one-hot (bias port)
                hi_neg = cpool.tile([P, RPC * STILES], F32, tag="hi_neg",
                                    bufs=2)
                nc.vector.tensor_scalar(
                    out=hi_neg[:], in0=hi_f[:], scalar1=-1.0, scalar2=None,
                    op0=mybir.AluOpType.mult)

                # ---------- per-row histograms -> a = H^2 * idfT ----------
                a_all = [cpool.tile([P, NHI, RPC], F32, name=f"a_all{mh}",
                                    tag=f"a_all{mh}", bufs=2)
                         for mh in range(2)]
                if phase == 2:
                    # AllToAll buffers: [hi(200=8sh*25), mh, p, r]
                    a2a_in = dpool.tile([8 * NHL, 2, P, RPC], F32,
                                        tag="a2a_in")
                    a2a_out = dpool.tile([8, NHL, 2, P, RPC], F32,
                                         tag="a2a_out")
                    # zero the hi 196..199 pad region (uninit DRAM garbage
                    # would ride the AllToAll and NaN-poison the matmul)
                    zpad = cpool.tile([P, (8 * NHL - NHI) * 2 * RPC], F32,
                                      tag="zpad", bufs=2)
                    nc.vector.memset(zpad[:], 0.0)
                    nc.sync.dma_start(
                        a2a_in[NHI:, :, :, :].rearrange(
                            "hi mh p r -> p hi mh r"),
                        zpad[:].rearrange("p (hi mh r) -> p hi mh r",
                                          hi=8 * NHL - NHI, mh=2),
                    )

                with tc.tile_pool(name="ps_ht", bufs=2, space="PSUM") as ps_ht:
                    for r in range(RPC):
                        ht_ps = [ps_ht.tile([P, NHI], F32, name=f"ht{mh}",
                                            tag=f"ht{mh}")
                                 for mh in range(2)]
                        for f in range(STILES):
                            col = r * STILES + f
                            lo_oh = ohpool.tile([P, NLO], BF16, tag="lo_oh")
                            hi_oh = ohpool.tile([P, NHI], BF16, tag="hi_oh")
                            if True:
                                nc.vector.tensor_scalar(
                                    out=lo_oh[:], in0=iota_bf[:],
                                    scalar1=lo_f[:, col : col + 1],
                                    scalar2=None,
                                    op0=mybir.AluOpType.is_equal)
                                d2 = ohpool.tile([P, NHI], F32,
                                                 tag="d2")
                                nc.scalar.activation(
                                    d2[:], iota_bf[:, :NHI],
                                    mybir.ActivationFunctionType.Square,
                                    bias=hi_neg[:, col : col + 1], scale=1.0)
                                nc.scalar.activation(
                                    hi_oh[:], d2[:],
                                    mybir.ActivationFunctionType.Relu,
                                    bias=1.0, scale=-1.0)
                            for mh in range(2):
                                nc.tensor.matmul(
                                    ht_ps[mh][:, :],
                                    lhsT=lo_oh[:, mh * P : (mh + 1) * P],
                                    rhs=hi_oh[:, :],
                                    start=(f == 0), stop=(f == STILES - 1))
                        for mh in range(2):
                            sq = wpool.tile([P, NHI], F32, tag="sq")
                            nc.scalar.square(sq[:], ht_ps[mh][:, :])
                            nc.vector.tensor_tensor(
                                out=a_all[mh][:, :, r], in0=sq[:],
                                in1=idf_sb[:, mh, :],
                                op=mybir.AluOpType.mult)

                if phase == 2:
                    # bulk-write a to the exchange buffer: one DMA per mh,
                    # innermost r contiguous (32B runs)
                    for mh in range(2):
                        nc.sync.dma_start(
                            a2a_in[:NHI, mh, :, :].rearrange(
                                "hi p r -> p hi r"),
                            a_all[mh][:, :, :],
                        )

                if phase == 1:
                    # ------- pooled over full-vocab chunks (M=RPC) -------
                    pooled_ps = ps_acc.tile([RPC, D], F32, tag="pooled")
                    chunks = []
                    v = 0
                    while v + 1024 <= 49152:
                        chunks.append((v, 1024))
                        v += 1024
                    chunks.append((49152, 768))
                    chunks.append((49920, 80))
                    n_mm = sum((n + 127) // 128 for _, n in chunks)
                    mm_i = 0
                    for v0, n in chunks:
                        embc = embpool.tile([P, 8, D], F32, tag="embc")
                        if n >= P:
                            nsub = n // P
                            nc.sync.dma_start(
                                embc[:, :nsub, :],
                                emb[v0 : v0 + nsub * P, :].rearrange(
                                    "(c p) d -> p c d", p=P))
                        else:
                            nsub = 1
                            nc.sync.dma_start(embc[:n, 0, :],
                                              emb[v0 : v0 + n, :])
                        for c in range(nsub):
                            vv = v0 + c * P
                            kk = min(P, n - c * P)
                            hi = vv >> 8
                            mh = (vv >> 7) & 1
                            nc.tensor.matmul(
                                pooled_ps[:, :],
                                lhsT=a_all[mh][:kk, hi, :],
                                rhs=embc[:kk, c, :],
                                start=(mm_i == 0), stop=(mm_i == n_mm - 1))
                            mm_i += 1
                    pooled_sb = cpool.tile([RPC, D], F32, tag="pooled_sb")
                    nc.vector.tensor_copy(pooled_sb[:], pooled_ps[:, :])
                else:
                    # ------- AllToAll, shard matmul (M=64), ReduceScatter ---
                    nc.gpsimd.collective_compute(
                        "AllToAll", mybir.AluOpType.bypass,
                        replica_groups=[list(range(NCORES))],
                        ins=[a2a_in[:, :, :, :]],
                        outs=[a2a_out[:, :, :, :, :]],
                    )
                    pooled_ps = ps_acc.tile([B, D], F32, tag="pooled")
                    # emb shard chunks of 1024 rows (6 full + 1x256)
                    ech = [(i * 1024, 1024) for i in range(6)] + [(6144, 256)]
                    mm_i = 0
                    for v0, n in ech:
                        embc = embpool.tile([P, 8, D], F32, tag="embc")
                        nsub = n // P
                        nc.sync.dma_start(
                            embc[:, :nsub, :],
                            embs[v0 : v0 + n, :].rearrange(
                                "(c p) d -> p c d", p=P))
                        for c in range(nsub):
                            k = v0 // P + c  # 0..49
                            hl, mh = k >> 1, k & 1
                            ar = arpool.tile([P, B], F32, tag="ar")
                            nc.sync.dma_start(
                                ar[:].rearrange("p (j r) -> p j r", j=8),
                                a2a_out[:, hl, mh, :, :].rearrange(
                                    "j p r -> p j r"))
                            nc.tensor.matmul(
                                pooled_ps[:, :], lhsT=ar[:],
                                rhs=embc[:, c, :],
                                start=(mm_i == 0), stop=(mm_i == 49))
                            mm_i += 1
                    pooled_full = cpool.tile([B, D], F32, tag="pooled_full",
                                             bufs=2)
                    nc.vector.tensor_copy(pooled_full[:], pooled_ps[:, :])
                    rs_in = dpool.tile([B, D], F32, tag="rs_in")
                    rs_out = dpool.tile([RPC, D], F32, tag="rs_out")
                    nc.sync.dma_start(rs_in[:, :], pooled_full[:])
                    nc.gpsimd.collective_compute(
                        "ReduceScatter", mybir.AluOpType.add,
                        replica_groups=[list(range(NCORES))],
                        ins=[rs_in[:, :]],
                        outs=[rs_out[:, :]],
                    )
                    pooled_sb = cpool.tile([RPC, D], F32, tag="pooled_sb")
                    nc.sync.dma_start(pooled_sb[:], rs_out[:, :])

                # ---------- MLP + softmax on own 8 rows ----------
                with tc.tile_pool(name="ps_mlp", bufs=1,
                                  space="PSUM") as ps_mlp:
                    _mlp_tail(nc, tc, cpool, ps_mlp, pooled_sb, identity,
                              w1t_sb, b1_sb, w2t_sb, b2a_sb, b2b_sb,
                              w3a_sb, w3b_sb, b3_sb, out)

    nc.compile()
    return nc


def _get_nc(phase=1, reps=1):
    key = f"nc_p{phase}_r{reps}"
    if key not in _CACHE:
        _CACHE[key] = _build_nc(phase, reps)
    return _CACHE[key]


class _Runner:
    """Cached jitted shard_map over the NEFF custom call (mirrors
    bass2jax.run_bass_via_pjrt, but reusable with device-resident inputs)."""

    def __init__(self, nc):
        import jax
        from jax.experimental.shard_map import shard_map
        from jax.sharding import Mesh, NamedSharding, PartitionSpec

        from concourse import bass2jax

        bass2jax.install_neuronx_cc_hook()
        assert nc.dbg_addr is None
        partition_name = (
            nc.partition_id_tensor.name if nc.partition_id_tensor else None
        )
        self._nc = nc
        self._partition_name = partition_name

        self.jax = jax
        in_names, out_names, out_avals, zero_outs = [], [], [], []
        for alloc in nc.m.functions[0].allocations:
            if not isinstance(alloc, mybir.MemoryLocationSet):
                continue
            name = alloc.memorylocations[0].name
            if alloc.kind == "ExternalInput":
                if name == partition_name:
                    continue
                in_names.append(name)
            elif alloc.kind == "ExternalOutput":
                out_names.append(name)
                shape = tuple(alloc.tensor_shape)
                dtype = mybir.dt.np(alloc.dtype)
                out_avals.append(jax.core.ShapedArray(shape, dtype))
                zero_outs.append(np.zeros((NCORES * shape[0], *shape[1:]), dtype))
        self.in_names = list(in_names)
        self.out_names = out_names
        self.out_avals = out_avals
        self.zero_outs = zero_outs
        n_params = len(in_names)
        n_outs = len(out_names)
        bind_names = tuple(
            in_names + out_names + ([partition_name] if partition_name else [])
        )
        donate = tuple(range(n_params, n_params + n_outs))

        def _body(*args):
            operands = list(args)
            if partition_name is not None:
                operands.append(bass2jax.partition_id_tensor())
            outs = bass2jax._bass_exec_p.bind(
                *operands,
                out_avals=tuple(out_avals),
                in_names=bind_names,
                out_names=tuple(out_names),
                lowering_input_output_aliases=(),
                sim_require_finite=True,
                sim_require_nnan=True,
                nc=nc,
            )
            return tuple(outs)

        devices = jax.devices()[:NCORES]
        self.mesh = Mesh(np.asarray(devices), ("core",))
        self.sharding = NamedSharding(self.mesh, PartitionSpec("core"))
        in_specs = (PartitionSpec("core"),) * (n_params + n_outs)
        out_specs = (PartitionSpec("core"),) * n_outs
        self.fn = jax.jit(
            shard_map(
                _body,
                mesh=self.mesh,
                in_specs=in_specs,
                out_specs=out_specs,
                check_rep=False,
            ),
            donate_argnums=donate,
            keep_unused=True,
        )

    def put_inputs(self, in_maps):
        concat = [
            np.concatenate([np.asarray(m[name]) for m in in_maps], axis=0)
            for name in self.in_names
        ]
        return [self.jax.device_put(a, self.sharding) for a in concat]

    def run(self, dev_in):
        zo = [self.jax.device_put(z, self.sharding) for z in self.zero_outs]
        outs = self.fn(*dev_in, *zo)
        self.jax.block_until_ready(outs)
        return outs

    def run_np(self, dev_in):
        outs = self.run(dev_in)
        return {
            name: np.asarray(outs[i]).reshape(NCORES, *self.out_avals[i].shape)
            for i, name in enumerate(self.out_names)
        }


def _get_runner(phase=None, reps=1):
    if phase is None:
        phase = int(os.environ.get("KERNEL_PHASE", "1"))
    key = f"runner_p{phase}_r{reps}"
    if key not in _CACHE:
        _CACHE[key] = _Runner(_get_nc(phase, reps))
    return _CACHE[key]


def make_in_maps(x, emb, idf, W1, b1, W2, b2, W3, b3, phase):
    xt = np.ascontiguousarray(np.asarray(x, dtype=np.int32).T)  # [B, S]
    emb = np.ascontiguousarray(np.asarray(emb, dtype=np.float32))
    idf = np.asarray(idf, dtype=np.float32)
    if phase == 3:
        idf_t = None  # idf folded into the emb shards
    else:
        idf_pad = np.zeros(VPAD, dtype=np.float32)
        idf_pad[:V] = idf
        idf_pad[0] = 0.0  # pad token contributes nothing
        idf_t = np.ascontiguousarray(idf_pad.reshape(NHI, NLO).T)  # [256, 196]

    w1t = np.ascontiguousarray(np.asarray(W1, dtype=np.float32).T)
    w2t = np.ascontiguousarray(np.asarray(W2, dtype=np.float32).T)
    w3t = np.ascontiguousarray(np.asarray(W3, dtype=np.float32).T)
    b1 = np.ascontiguousarray(np.asarray(b1, dtype=np.float32))
    b2 = np.ascontiguousarray(np.asarray(b2, dtype=np.float32))
    b3 = np.ascontiguousarray(np.asarray(b3, dtype=np.float32))

    if phase in (2, 3):
        emb_pad = np.zeros((NCORES * VSH, D), dtype=np.float32)
        emb_pad[:V] = emb
        if phase == 3:
            import ml_dtypes
            idf_z = np.zeros(NCORES * VSH, dtype=np.float32)
            idf_z[:V] = idf
            idf_z[0] = 0.0  # pad token contributes nothing
            emb_pad = (emb_pad * idf_z[:, None]).astype(ml_dtypes.bfloat16)
    in_maps = []
    for c in range(NCORES):
        m = {
            "xt": np.ascontiguousarray(xt[c * RPC : (c + 1) * RPC, :]),
            "w1t": w1t, "b1": b1, "w2t": w2t, "b2": b2,
            "w3t": w3t, "b3": b3,
        }
        if phase != 3:
            m["idf_t"] = idf_t
        if phase == 1:
            m["emb"] = emb
        else:
            m["embs"] = np.ascontiguousarray(emb_pad[c * VSH : (c + 1) * VSH])
        in_maps.append(m)
    return in_maps


def kernel(x, emb, idf, W1, b1, W2, b2, W3, b3):
    phase = int(os.environ.get("KERNEL_PHASE", "1"))
    in_maps = make_in_maps(x, emb, idf, W1, b1, W2, b2, W3, b3, phase)
    runner = _get_runner(phase)
    dev_in = runner.put_inputs(in_maps)
    _CACHE["last_dev_in"] = dev_in
    outs = runner.run_np(dev_in)
    outp = np.concatenate([outs["out"][c] for c in range(NCORES)], axis=0)
    return outp.astype(np.float32)

